# revision 22
# baseline (speedup 1.0000x reference)
"""Causal self-attention (GQA + RoPE) Trainium2 Bass kernel, 8 NeuronCores.

Problem: B=2, T=2048, C=2048, n_head=16, n_kv_head=4, head_dim=128.

Sharding: 2-way batch DP x 4-way head TP. Core c = 4*b + g handles batch b,
kv head g, q heads [4g, 4g+4). wq/wk/wv column-sharded per head group, wo
row-sharded; per-core partial outputs are summed on the host (the gather /
unshard step), so no on-device collective is needed.

v2: fully fused pipeline. Per 512-col t-chunk i the PE emission order is
  proj(i) -> V-transpose(i) -> outproj(i-1) -> attention(i)
so the tensor engine never crosses a phase barrier (keeps HAM warm).
Projection runs one output at a time (k, q0..q3, v: 16-chunk accumulation
chains in a single PSUM bank each) so projections need only the 2 shared
"generic" PSUM banks; attention uses 2-bank score pairs (one exp per block
pair), accumulating den (ones-matmul) and O over s-blocks; softmax denom
reciprocal via the fast approx DVE op. All DRAM inputs are host-pre-tiled
to match SBUF layouts so every DMA is contiguous; outputs are fp16
partials summed on the host.
"""

import sys

sys.path.insert(0, "/opt/trn_rl_repo")

import numpy as np

import concourse.bass as bass
import concourse.mybir as mybir
import concourse.tile as tile
from concourse import bacc
from concourse.bass_utils import run_bass_kernel_spmd
from concourse.masks import make_identity

F32 = mybir.dt.float32
F16 = mybir.dt.float16
AF = mybir.ActivationFunctionType

B, T, C = 2, 2048, 2048
N_HEAD, N_KV_HEAD = 16, 4
HD = 128                 # head dim
QH = 4                   # q heads per core
TQ = 512                 # t-chunk
NT = T // TQ             # 4 t-chunks
CK = C // 128            # 16 contraction chunks of 128
SCALE = 1.0 / float(np.sqrt(HD))
MASK_NEG = -1e30

_CACHE = {}


def _build_nc():
    nc = bacc.Bacc("TRN2", target_bir_lowering=False, debug=False, num_devices=8)

    # All inputs pre-tiled on host so DRAM layout == SBUF layout.
    xH = nc.dram_tensor("xH", [NT, 128, CK, TQ], F16, kind="ExternalInput").ap()
    wqH = nc.dram_tensor("wqH", [128, CK, QH * HD], F16, kind="ExternalInput").ap()
    wkH = nc.dram_tensor("wkH", [128, CK, HD], F16, kind="ExternalInput").ap()
    wvH = nc.dram_tensor("wvH", [128, CK, HD], F16, kind="ExternalInput").ap()
    woH = nc.dram_tensor("woH", [128, CK, QH * HD], F16, kind="ExternalInput").ap()
    cosH = nc.dram_tensor("cosH", [HD, T], F16, kind="ExternalInput").ap()
    sinH = nc.dram_tensor("sinH", [HD, T], F16, kind="ExternalInput").ap()
    outX = nc.dram_tensor("outX", [NT, 128, CK, TQ], F16, kind="ExternalOutput").ap()

    with tile.TileContext(nc) as tc:
        _emit(nc, tc, xH, wqH, wkH, wvH, woH, cosH, sinH, outX)

    nc.compile()
    return nc


def _emit(nc, tc, xH, wqH, wkH, wvH, woH, cosH, sinH, outX):
    import contextlib

    ctx = contextlib.ExitStack()
    with ctx:
        singles = ctx.enter_context(tc.tile_pool(name="singles", bufs=1))

        # ---- resident tiles ----
        wq_sb = singles.tile([128, CK, QH * HD], F16)
        wk_sb = singles.tile([128, CK, HD], F16)
        wv_sb = singles.tile([128, CK, HD], F16)
        wo_sb = singles.tile([128, CK, QH * HD], F16)
        cos_sb = singles.tile([HD, T], F16)
        sin_sb = singles.tile([HD, T], F16)

        qT_sb = singles.tile([128, QH, T], F16)    # per head [dq, t], RoPE'd
        kT_sb = singles.tile([128, T], F16)        # [dk, t], RoPE'd
        v_sb = singles.tile([128, CK, HD], F16)    # [s in blk, (blk, dv)]
        oT_sb = singles.tile([128, QH, T], F16)    # per head [dv, t] normalized

        ident = singles.tile([128, 128], F32)
        cmask = singles.tile([128, 128], F32)
        ones_sq = singles.tile([128, 128], F16)

        # ---- pools ----
        xpool = ctx.enter_context(tc.tile_pool(name="xpool", bufs=2))
        ppool = ctx.enter_context(tc.tile_pool(name="ppool", bufs=4))
        vtsb = ctx.enter_context(tc.tile_pool(name="vtsb", bufs=2))
        rope = ctx.enter_context(tc.tile_pool(name="rope", bufs=4))
        invp = ctx.enter_context(tc.tile_pool(name="invp", bufs=2))
        outsb = ctx.enter_context(tc.tile_pool(name="outsb", bufs=2))
        ps_s = ctx.enter_context(tc.tile_pool(name="ps_s", bufs=2, space="PSUM"))
        ps_d = ctx.enter_context(tc.tile_pool(name="ps_d", bufs=1, space="PSUM"))
        ps_o = ctx.enter_context(tc.tile_pool(name="ps_o", bufs=1, space="PSUM"))
        ps_g = ctx.enter_context(tc.tile_pool(name="ps_g", bufs=2, space="PSUM"))

        # ---- startup DMAs, criticality-ordered ----
        # x chunk 0 sub-DMA a=0 first so the first matmul starts ASAP;
        # weights interleaved so chunk k arrives before its matmul.
        x_t = [None] * NT

        def load_x(i):
            x_t[i] = xpool.tile([128, CK, TQ], F16, tag="x", name=f"x{i}")
            for a in range(4):
                nc.sync.dma_start(out=x_t[i][:, 4 * a:4 * a + 4, :],
                                  in_=xH[i, :, 4 * a:4 * a + 4, :])

        # interleave wq with x0 so each arrives just before its matmuls
        nc.sync.dma_start(out=wk_sb, in_=wkH)
        x_t[0] = xpool.tile([128, CK, TQ], F16, tag="x", name="x0")
        for a in range(4):
            nc.sync.dma_start(out=x_t[0][:, 4 * a:4 * a + 4, :],
                              in_=xH[0, :, 4 * a:4 * a + 4, :])
            nc.sync.dma_start(out=wq_sb[:, 4 * a:4 * a + 4, :],
                              in_=wqH[:, 4 * a:4 * a + 4, :])
        nc.sync.dma_start(out=cos_sb, in_=cosH)
        nc.sync.dma_start(out=sin_sb, in_=sinH)
        nc.sync.dma_start(out=wv_sb, in_=wvH)
        load_x(1)
        for a in range(2):
            nc.sync.dma_start(out=wo_sb[:, 8 * a:8 * a + 8, :],
                              in_=woH[:, 8 * a:8 * a + 8, :])

        make_identity(nc, ident)
        nc.gpsimd.memset(cmask, 0.0)
        nc.gpsimd.affine_select(
            out=cmask, in_=cmask, compare_op=mybir.AluOpType.is_ge,
            fill=MASK_NEG, base=0, pattern=[[1, 128]], channel_multiplier=-1,
        )
        nc.vector.memset(ones_sq, 1.0)

        # HAM pre-warm: ~7us of dummy matmuls covering the initial DMA ramp,
        # so real matmuls run at 2.4 GHz from the start with no re-throttle.
        for w in range(2):
            warm = ps_g.tile([128, 128], F32, tag="g", name=f"warm{w}")
            for _ in range(40):
                nc.tensor.matmul(warm, ones_sq, ones_sq, start=True, stop=True)

        def do_rope(tgt, ti):
            """In-place RoPE on tgt ([128, TQ] slice, f16)."""
            sw = rope.tile([128, TQ], F16, tag="swap")
            nc.sync.dma_start(out=sw[0:64, :], in_=tgt[64:128, :])
            nc.sync.dma_start(out=sw[64:128, :], in_=tgt[0:64, :])
            tmp = rope.tile([128, TQ], F16, tag="tmp")
            nc.vector.tensor_mul(tmp, tgt, cos_sb[:, ti:ti + TQ])
            nc.vector.tensor_mul(sw, sw, sin_sb[:, ti:ti + TQ])
            nc.vector.tensor_add(tgt, tmp, sw)

        def proj_pass(i, w_sb, col0, ncol, kind, h=None):
            """One projection output over all 16 c-chunks into 1 PSUM bank."""
            ti = TQ * i
            acc = ps_g.tile([128, TQ], F32, tag="g")
            for kk in range(CK):
                nc.tensor.matmul(acc, w_sb[:, kk, col0:col0 + ncol],
                                 x_t[i][:, kk, :],
                                 start=(kk == 0), stop=(kk == CK - 1))
            if kind == "k":
                nc.vector.tensor_copy(out=kT_sb[:, ti:ti + TQ], in_=acc)
                do_rope(kT_sb[:, ti:ti + TQ], ti)
                return None
            if kind == "q":
                nc.vector.tensor_copy(out=qT_sb[:, h, ti:ti + TQ], in_=acc)
                do_rope(qT_sb[:, h, ti:ti + TQ], ti)
                return None
            vt = vtsb.tile([128, TQ], F32, tag="vt")
            nc.vector.tensor_copy(out=vt, in_=acc)
            return vt

        def vts(i, vt):
            """V^T -> natural [s, dv] blocks via PE transpose."""
            for jj in range(TQ // 128):
                vt_ps = ps_g.tile([128, 128], F32, tag="g")
                nc.tensor.transpose(vt_ps, vt[:, 128 * jj:128 * (jj + 1)], ident)
                nc.vector.tensor_copy(out=v_sb[:, 4 * i + jj, :], in_=vt_ps)

        def outproj_units(i):
            """16 co-block emitters for output projection of t-chunk i;
            used as PE gap-filler inside the next chunk's attention."""
            ti = TQ * i
            osb = outsb.tile([128, CK, TQ], F16, tag="ot", name=f"osb{i}")

            def unit(co):
                def emit():
                    ot = ps_g.tile([128, TQ], F32, tag="g")
                    for h in range(QH):
                        nc.tensor.matmul(ot, wo_sb[:, co, HD * h:HD * (h + 1)],
                                         oT_sb[:, h, ti:ti + TQ],
                                         start=(h == 0), stop=(h == QH - 1))
                    nc.vector.tensor_copy(out=osb[:, co, :], in_=ot)
                    if co % 4 == 3:  # store per 4-co group
                        a = co // 4
                        nc.sync.dma_start(out=outX[i, :, 4 * a:4 * a + 4, :],
                                          in_=osb[:, 4 * a:4 * a + 4, :])
                return emit
            return [unit(co) for co in range(CK)]

        def proj_units(i, passes):
            """Projection passes split into 4-matmul chain steps, usable as
            attention gap-filler. passes: list of (kind, h)."""
            units = []
            for kind, h in passes:
                if kind == "k":
                    w, col0 = wk_sb, 0
                elif kind == "v":
                    w, col0 = wv_sb, 0
                else:
                    w, col0 = wq_sb, HD * h
                acc = ps_g.tile([128, TQ], F32, tag="g",
                                name=f"acc{i}_{kind}{h}")

                def step(acc, w, col0, g0):
                    def emit():
                        for kk in range(g0, g0 + 4):
                            nc.tensor.matmul(acc, w[:, kk, col0:col0 + HD],
                                             x_t[i][:, kk, :],
                                             start=(kk == 0),
                                             stop=(kk == CK - 1))
                    return emit

                def evac(acc, kind, h, ti):
                    def emit():
                        if kind == "k":
                            nc.vector.tensor_copy(out=kT_sb[:, ti:ti + TQ],
                                                  in_=acc)
                            do_rope(kT_sb[:, ti:ti + TQ], ti)
                        else:
                            nc.vector.tensor_copy(out=qT_sb[:, h, ti:ti + TQ],
                                                  in_=acc)
                            do_rope(qT_sb[:, h, ti:ti + TQ], ti)
                    return emit

                for g0 in range(0, CK, 4):
                    units.append(step(acc, w, col0, g0))
                units.append(evac(acc, kind, h, TQ * i))
            return units

        def attn_chunk(i, fillers):
            """Attention for t-chunk i: flat pair-stream over (head, pair)
            with one-item lookahead so PE rarely waits on exp; `fillers`
            (outproj co-blocks or next-chunk proj steps) are interleaved
            evenly to cover exp latency with useful matmuls."""
            ti = TQ * i
            nj = 4 * (i + 1)
            npair = nj // 2
            n_items = QH * npair
            fill_idx = 0

            def blk(j):
                t0 = max(ti, 128 * j)
                return t0, TQ * (i + 1) - t0, t0 - ti  # t0, N, c0

            acc_t = {}  # h -> (den, o_ps)

            def flush(h, p, pp, blocks):
                if p == 0:
                    den = ps_d.tile([128, TQ], F32, tag="d",
                                    name=f"den{i}_{h}")
                    o_ps = ps_o.tile([128, TQ], F32, tag="o",
                                     name=f"o{i}_{h}")
                    acc_t[h] = (den, o_ps)
                den, o_ps = acc_t[h]
                first, last = (p == 0), (p == npair - 1)
                for bi, (j, loc, N, c0) in enumerate(blocks):
                    st = first and bi == 0
                    sp = last and bi == len(blocks) - 1
                    nc.tensor.matmul(den[:, c0:c0 + N], ones_sq,
                                     pp[:, loc:loc + N], start=st, stop=sp)
                for bi, (j, loc, N, c0) in enumerate(blocks):
                    st = first and bi == 0
                    sp = last and bi == len(blocks) - 1
                    nc.tensor.matmul(o_ps[:, c0:c0 + N], v_sb[:, j, :],
                                     pp[:, loc:loc + N], start=st, stop=sp)
                if last:
                    inv = invp.tile([128, TQ], F32, tag="inv")
                    nc.vector.reciprocal_approx_fast(out=inv, in_=den)
                    nc.vector.tensor_mul(oT_sb[:, h, ti:ti + TQ], o_ps, inv)

            pend = None
            n = 0
            for h in range(QH):
                for p in range(npair):
                    j0, j1 = 2 * p, 2 * p + 1
                    t0a, N0, c0a = blk(j0)
                    t0b, N1, c0b = blk(j1)
                    sp_t = ps_s.tile([128, 2 * TQ], F32, tag="s")
                    nc.tensor.matmul(sp_t[:, 0:N0],
                                     kT_sb[:, 128 * j0:128 * (j0 + 1)],
                                     qT_sb[:, h, t0a:t0a + N0],
                                     start=True, stop=True)
                    nc.tensor.matmul(sp_t[:, TQ:TQ + N1],
                                     kT_sb[:, 128 * j1:128 * (j1 + 1)],
                                     qT_sb[:, h, t0b:t0b + N1],
                                     start=True, stop=True)
                    if j0 >= 4 * i:  # diagonal blocks: causal mask
                        nc.vector.tensor_add(sp_t[:, 0:128],
                                             sp_t[:, 0:128], cmask)
                    if j1 >= 4 * i:
                        nc.vector.tensor_add(sp_t[:, TQ:TQ + 128],
                                             sp_t[:, TQ:TQ + 128], cmask)
                    pp = ppool.tile([128, 2 * TQ], F16, tag="p")
                    ncols = TQ + N1
                    nc.scalar.activation(pp[:, :ncols], sp_t[:, :ncols],
                                         AF.Exp, scale=SCALE)
                    if pend is not None:
                        flush(*pend)
                    pend = (h, p, pp, [(j0, 0, N0, c0a), (j1, TQ, N1, c0b)])
                    n += 1
                    while fill_idx * n_items < n * len(fillers):
                        fillers[fill_idx]()
                        fill_idx += 1
            flush(*pend)
            while fill_idx < len(fillers):
                fillers[fill_idx]()
                fill_idx += 1

        def full_proj(i):
            """All projections for chunk i, V-transposes mid-way so they
            don't queue behind all the RoPE work on the DVE."""
            proj_pass(i, wk_sb, 0, HD, "k")
            proj_pass(i, wq_sb, 0, HD, "q", h=0)
            proj_pass(i, wq_sb, HD, HD, "q", h=1)
            vt = proj_pass(i, wv_sb, 0, HD, "v")
            vts(i, vt)
            proj_pass(i, wq_sb, 2 * HD, HD, "q", h=2)
            proj_pass(i, wq_sb, 3 * HD, HD, "q", h=3)

        # ======== fused pipeline ========
        # chunk 0 projections, then attn(0) filled with proj(1) k/q0 steps,
        # then the rest of proj(1), then attn(i) filled with outproj(i-1).
        full_proj(0)
        attn_chunk(0, proj_units(1, [("k", None), ("q", 0)]))
        proj_pass(1, wq_sb, HD, HD, "q", h=1)
        vt = proj_pass(1, wv_sb, 0, HD, "v")
        vts(1, vt)
        proj_pass(1, wq_sb, 2 * HD, HD, "q", h=2)
        proj_pass(1, wq_sb, 3 * HD, HD, "q", h=3)
        load_x(2)
        attn_chunk(1, outproj_units(0))
        full_proj(2)
        load_x(3)
        attn_chunk(2, outproj_units(1))
        full_proj(3)
        attn_chunk(3, outproj_units(2))
        for u in outproj_units(3):
            u()


_PERM = np.concatenate([np.arange(0, HD, 2), np.arange(1, HD, 2)])

PROFILE = False
LAST_EXEC_NS = None
LAST_RESULTS = None


def kernel(x, freqs_cos, freqs_sin, wq, wk, wv, wo):
    global LAST_EXEC_NS, LAST_RESULTS
    if "nc" not in _CACHE:
        _CACHE["nc"] = _build_nc()
    nc = _CACHE["nc"]

    x = np.asarray(x, dtype=np.float32)
    fc = np.asarray(freqs_cos, dtype=np.float32)
    fs = np.asarray(freqs_sin, dtype=np.float32)
    wq = np.asarray(wq, dtype=np.float32)
    wk = np.asarray(wk, dtype=np.float32)
    wv = np.asarray(wv, dtype=np.float32)
    wo = np.asarray(wo, dtype=np.float32)

    cosT = fc.T                                   # [64, T]
    sinT = fs.T
    cosH = np.ascontiguousarray(
        np.concatenate([cosT, cosT], axis=0).astype(np.float16))   # [128, T]
    sinH = np.ascontiguousarray(
        np.concatenate([-sinT, sinT], axis=0).astype(np.float16))

    in_maps = []
    for core in range(8):
        b, g = core // 4, core % 4
        xT = x[b].T.astype(np.float16)                        # [C, T]
        # [C, T] -> [NT, 128(p), CK(k), TQ]: xH[i, p, k, t] = xT[128k+p, 512i+t]
        xH = np.ascontiguousarray(
            xT.reshape(CK, 128, NT, TQ).transpose(2, 1, 0, 3))
        wq_g = wq[512 * g:512 * (g + 1)].reshape(QH, HD, C)[:, _PERM, :]
        wqT = wq_g.reshape(QH * HD, C).T.astype(np.float16)   # [C, 512]
        wqH = np.ascontiguousarray(
            wqT.reshape(CK, 128, QH * HD).transpose(1, 0, 2))  # [128, CK, 512]
        wkT = wk[HD * g:HD * (g + 1)][_PERM].T.astype(np.float16)  # [C, 128]
        wkH = np.ascontiguousarray(wkT.reshape(CK, 128, HD).transpose(1, 0, 2))
        wvT = wv[HD * g:HD * (g + 1)].T.astype(np.float16)
        wvH = np.ascontiguousarray(wvT.reshape(CK, 128, HD).transpose(1, 0, 2))
        wo_g = wo[:, 512 * g:512 * (g + 1)]                   # [C, 512]
        # woH[p, co, 128h+d] = wo[128co+d, 512g+128h+p]
        woH = np.ascontiguousarray(
            wo_g.reshape(CK, 128, QH, 128).transpose(3, 0, 2, 1)
        ).astype(np.float16).reshape(128, CK, QH * 128)
        in_maps.append({
            "xH": xH, "wqH": wqH, "wkH": wkH, "wvH": wvH, "woH": woH,
            "cosH": cosH, "sinH": sinH,
        })

    res = run_bass_kernel_spmd(nc, in_maps, list(range(8)), trace=PROFILE)
    LAST_EXEC_NS = res.exec_time_ns
    LAST_RESULTS = res

    out = np.empty((B, T, C), dtype=np.float32)
    for b in range(B):
        acc = res.results[4 * b]["outX"].astype(np.float32)
        for g in range(1, 4):
            acc = acc + res.results[4 * b + g]["outX"]
        # outX[i, d?, co, t]: out[b][512i+t, 128co+d] = outX[i, d, co, t]
        out[b] = acc.transpose(0, 3, 2, 1).reshape(T, C)
    return out


# revision 24
# speedup vs baseline: 1.0049x; 1.0049x over previous
"""Causal self-attention (GQA + RoPE) Trainium2 Bass kernel, 8 NeuronCores.

Problem: B=2, T=2048, C=2048, n_head=16, n_kv_head=4, head_dim=128.

Sharding: 2-way batch DP x 4-way head TP. Core c = 4*b + g handles batch b,
kv head g, q heads [4g, 4g+4). wq/wk/wv column-sharded per head group, wo
row-sharded; per-core partial outputs are summed on the host (the gather /
unshard step), so no on-device collective is needed.

v2: fully fused pipeline. Per 512-col t-chunk i the PE emission order is
  proj(i) -> V-transpose(i) -> outproj(i-1) -> attention(i)
so the tensor engine never crosses a phase barrier (keeps HAM warm).
Projection runs one output at a time (k, q0..q3, v: 16-chunk accumulation
chains in a single PSUM bank each) so projections need only the 2 shared
"generic" PSUM banks; attention uses 2-bank score pairs (one exp per block
pair), accumulating den (ones-matmul) and O over s-blocks; softmax denom
reciprocal via the fast approx DVE op. All DRAM inputs are host-pre-tiled
to match SBUF layouts so every DMA is contiguous; outputs are fp16
partials summed on the host.
"""

import sys

sys.path.insert(0, "/opt/trn_rl_repo")

import numpy as np

import concourse.bass as bass
import concourse.mybir as mybir
import concourse.tile as tile
from concourse import bacc
from concourse.bass_utils import run_bass_kernel_spmd
from concourse.masks import make_identity

F32 = mybir.dt.float32
F16 = mybir.dt.float16
AF = mybir.ActivationFunctionType

B, T, C = 2, 2048, 2048
N_HEAD, N_KV_HEAD = 16, 4
HD = 128                 # head dim
QH = 4                   # q heads per core
TQ = 512                 # t-chunk
NT = T // TQ             # 4 t-chunks
CK = C // 128            # 16 contraction chunks of 128
SCALE = 1.0 / float(np.sqrt(HD))
MASK_NEG = -1e30

_CACHE = {}


def _build_nc():
    nc = bacc.Bacc("TRN2", target_bir_lowering=False, debug=False, num_devices=8)

    # All inputs pre-tiled on host so DRAM layout == SBUF layout.
    xH = nc.dram_tensor("xH", [NT, 128, CK, TQ], F16, kind="ExternalInput").ap()
    wqH = nc.dram_tensor("wqH", [128, CK, QH * HD], F16, kind="ExternalInput").ap()
    wkH = nc.dram_tensor("wkH", [128, CK, HD], F16, kind="ExternalInput").ap()
    wvH = nc.dram_tensor("wvH", [128, CK, HD], F16, kind="ExternalInput").ap()
    woH = nc.dram_tensor("woH", [128, CK, QH * HD], F16, kind="ExternalInput").ap()
    cosH = nc.dram_tensor("cosH", [HD, T], F16, kind="ExternalInput").ap()
    sinH = nc.dram_tensor("sinH", [HD, T], F16, kind="ExternalInput").ap()
    outX = nc.dram_tensor("outX", [NT, 128, CK, TQ], F16, kind="ExternalOutput").ap()

    with tile.TileContext(nc) as tc:
        _emit(nc, tc, xH, wqH, wkH, wvH, woH, cosH, sinH, outX)

    nc.compile()
    return nc


def _emit(nc, tc, xH, wqH, wkH, wvH, woH, cosH, sinH, outX):
    import contextlib

    ctx = contextlib.ExitStack()
    with ctx:
        singles = ctx.enter_context(tc.tile_pool(name="singles", bufs=1))

        # ---- resident tiles ----
        wq_sb = singles.tile([128, CK, QH * HD], F16)
        wk_sb = singles.tile([128, CK, HD], F16)
        wv_sb = singles.tile([128, CK, HD], F16)
        wo_sb = singles.tile([128, CK, QH * HD], F16)
        cos_sb = singles.tile([HD, T], F16)
        sin_sb = singles.tile([HD, T], F16)

        qT_sb = singles.tile([128, QH, T], F16)    # per head [dq, t], RoPE'd
        kT_sb = singles.tile([128, T], F16)        # [dk, t], RoPE'd
        v_sb = singles.tile([128, CK, HD], F16)    # [s in blk, (blk, dv)]
        oT_sb = singles.tile([128, QH, T], F16)    # per head [dv, t] normalized

        ident = singles.tile([128, 128], F32)
        cmask = singles.tile([128, 128], F32)
        ones_sq = singles.tile([128, 128], F16)

        # ---- pools ----
        xpool = ctx.enter_context(tc.tile_pool(name="xpool", bufs=2))
        ppool = ctx.enter_context(tc.tile_pool(name="ppool", bufs=3))
        vtsb = ctx.enter_context(tc.tile_pool(name="vtsb", bufs=2))
        rope = ctx.enter_context(tc.tile_pool(name="rope", bufs=3))
        invp = ctx.enter_context(tc.tile_pool(name="invp", bufs=2))
        outsb = ctx.enter_context(tc.tile_pool(name="outsb", bufs=2))
        ps_s = ctx.enter_context(tc.tile_pool(name="ps_s", bufs=2, space="PSUM"))
        ps_d = ctx.enter_context(tc.tile_pool(name="ps_d", bufs=1, space="PSUM"))
        ps_o = ctx.enter_context(tc.tile_pool(name="ps_o", bufs=1, space="PSUM"))
        ps_g = ctx.enter_context(tc.tile_pool(name="ps_g", bufs=2, space="PSUM"))

        # ---- startup DMAs, criticality-ordered ----
        # x chunk 0 sub-DMA a=0 first so the first matmul starts ASAP;
        # weights interleaved so chunk k arrives before its matmul.
        x_t = [None] * NT

        def load_x(i):
            x_t[i] = xpool.tile([128, CK, TQ], F16, tag="x", name=f"x{i}")
            for a in range(4):
                nc.sync.dma_start(out=x_t[i][:, 4 * a:4 * a + 4, :],
                                  in_=xH[i, :, 4 * a:4 * a + 4, :])

        # interleave wq with x0 so each arrives just before its matmuls;
        # x0 in 8 fine slices so the k-pass chain is never starved mid-run
        nc.sync.dma_start(out=wk_sb, in_=wkH)
        x_t[0] = xpool.tile([128, CK, TQ], F16, tag="x", name="x0")
        for a in range(4):
            nc.sync.dma_start(out=x_t[0][:, 4 * a:4 * a + 2, :],
                              in_=xH[0, :, 4 * a:4 * a + 2, :])
            nc.sync.dma_start(out=x_t[0][:, 4 * a + 2:4 * a + 4, :],
                              in_=xH[0, :, 4 * a + 2:4 * a + 4, :])
            nc.sync.dma_start(out=wq_sb[:, 4 * a:4 * a + 4, :],
                              in_=wqH[:, 4 * a:4 * a + 4, :])
        nc.sync.dma_start(out=cos_sb, in_=cosH)
        nc.sync.dma_start(out=sin_sb, in_=sinH)
        nc.sync.dma_start(out=wv_sb, in_=wvH)
        load_x(1)
        for a in range(2):
            nc.sync.dma_start(out=wo_sb[:, 8 * a:8 * a + 8, :],
                              in_=woH[:, 8 * a:8 * a + 8, :])

        make_identity(nc, ident)
        nc.gpsimd.memset(cmask, 0.0)
        nc.gpsimd.affine_select(
            out=cmask, in_=cmask, compare_op=mybir.AluOpType.is_ge,
            fill=MASK_NEG, base=0, pattern=[[1, 128]], channel_multiplier=-1,
        )
        nc.vector.memset(ones_sq, 1.0)

        # HAM pre-warm: ~7us of dummy matmuls covering the initial DMA ramp,
        # so real matmuls run at 2.4 GHz from the start with no re-throttle.
        for w in range(2):
            warm = ps_g.tile([128, 128], F32, tag="g", name=f"warm{w}")
            for _ in range(40):
                nc.tensor.matmul(warm, ones_sq, ones_sq, start=True, stop=True)

        def do_rope(tgt, ti):
            """In-place RoPE on tgt ([128, TQ] slice, f16)."""
            sw = rope.tile([128, TQ], F16, tag="swap")
            nc.sync.dma_start(out=sw[0:64, :], in_=tgt[64:128, :])
            nc.sync.dma_start(out=sw[64:128, :], in_=tgt[0:64, :])
            tmp = rope.tile([128, TQ], F16, tag="tmp")
            nc.vector.tensor_mul(tmp, tgt, cos_sb[:, ti:ti + TQ])
            nc.vector.tensor_mul(sw, sw, sin_sb[:, ti:ti + TQ])
            nc.vector.tensor_add(tgt, tmp, sw)

        def proj_pass(i, w_sb, col0, ncol, kind, h=None):
            """One projection output over all 16 c-chunks into 1 PSUM bank."""
            ti = TQ * i
            acc = ps_g.tile([128, TQ], F32, tag="g")
            for kk in range(CK):
                nc.tensor.matmul(acc, w_sb[:, kk, col0:col0 + ncol],
                                 x_t[i][:, kk, :],
                                 start=(kk == 0), stop=(kk == CK - 1))
            if kind == "k":
                nc.vector.tensor_copy(out=kT_sb[:, ti:ti + TQ], in_=acc)
                do_rope(kT_sb[:, ti:ti + TQ], ti)
                return None
            if kind == "q":
                nc.vector.tensor_copy(out=qT_sb[:, h, ti:ti + TQ], in_=acc)
                do_rope(qT_sb[:, h, ti:ti + TQ], ti)
                return None
            vt = vtsb.tile([128, TQ], F32, tag="vt")
            nc.vector.tensor_copy(out=vt, in_=acc)
            return vt

        def vts(i, vt):
            """V^T -> natural [s, dv] blocks via PE transpose."""
            for jj in range(TQ // 128):
                vt_ps = ps_g.tile([128, 128], F32, tag="g")
                nc.tensor.transpose(vt_ps, vt[:, 128 * jj:128 * (jj + 1)], ident)
                nc.vector.tensor_copy(out=v_sb[:, 4 * i + jj, :], in_=vt_ps)

        def outproj_units(i):
            """16 co-block emitters for output projection of t-chunk i;
            used as PE gap-filler inside the next chunk's attention."""
            ti = TQ * i
            osb = outsb.tile([128, CK, TQ], F16, tag="ot", name=f"osb{i}")

            def unit(co):
                def emit():
                    ot = ps_g.tile([128, TQ], F32, tag="g")
                    for h in range(QH):
                        nc.tensor.matmul(ot, wo_sb[:, co, HD * h:HD * (h + 1)],
                                         oT_sb[:, h, ti:ti + TQ],
                                         start=(h == 0), stop=(h == QH - 1))
                    nc.vector.tensor_copy(out=osb[:, co, :], in_=ot)
                    if co % 4 == 3:  # store per 4-co group
                        a = co // 4
                        nc.sync.dma_start(out=outX[i, :, 4 * a:4 * a + 4, :],
                                          in_=osb[:, 4 * a:4 * a + 4, :])
                return emit
            return [unit(co) for co in range(CK)]

        def proj_units(i, passes):
            """Projection passes split into 4-matmul chain steps, usable as
            attention gap-filler. passes: list of (kind, h)."""
            units = []
            for kind, h in passes:
                if kind == "k":
                    w, col0 = wk_sb, 0
                elif kind == "v":
                    w, col0 = wv_sb, 0
                else:
                    w, col0 = wq_sb, HD * h
                acc = ps_g.tile([128, TQ], F32, tag="g",
                                name=f"acc{i}_{kind}{h}")

                def step(acc, w, col0, g0):
                    def emit():
                        for kk in range(g0, g0 + 4):
                            nc.tensor.matmul(acc, w[:, kk, col0:col0 + HD],
                                             x_t[i][:, kk, :],
                                             start=(kk == 0),
                                             stop=(kk == CK - 1))
                    return emit

                def evac(acc, kind, h, ti):
                    def emit():
                        if kind == "k":
                            nc.vector.tensor_copy(out=kT_sb[:, ti:ti + TQ],
                                                  in_=acc)
                            do_rope(kT_sb[:, ti:ti + TQ], ti)
                        else:
                            nc.vector.tensor_copy(out=qT_sb[:, h, ti:ti + TQ],
                                                  in_=acc)
                            do_rope(qT_sb[:, h, ti:ti + TQ], ti)
                    return emit

                for g0 in range(0, CK, 4):
                    units.append(step(acc, w, col0, g0))
                units.append(evac(acc, kind, h, TQ * i))
            return units

        def attn_chunk(i, fillers):
            """Attention for t-chunk i: flat pair-stream over (head, pair)
            with one-item lookahead so PE rarely waits on exp; `fillers`
            (outproj co-blocks or next-chunk proj steps) are interleaved
            evenly to cover exp latency with useful matmuls."""
            ti = TQ * i
            nj = 4 * (i + 1)
            npair = nj // 2
            n_items = QH * npair
            fill_idx = 0

            def blk(j):
                t0 = max(ti, 128 * j)
                return t0, TQ * (i + 1) - t0, t0 - ti  # t0, N, c0

            acc_t = {}  # h -> (den, o_ps)

            def flush(h, p, pp, blocks):
                if p == 0:
                    den = ps_d.tile([128, TQ], F32, tag="d",
                                    name=f"den{i}_{h}")
                    o_ps = ps_o.tile([128, TQ], F32, tag="o",
                                     name=f"o{i}_{h}")
                    acc_t[h] = (den, o_ps)
                den, o_ps = acc_t[h]
                first, last = (p == 0), (p == npair - 1)
                for bi, (j, loc, N, c0) in enumerate(blocks):
                    st = first and bi == 0
                    sp = last and bi == len(blocks) - 1
                    nc.tensor.matmul(den[:, c0:c0 + N], ones_sq,
                                     pp[:, loc:loc + N], start=st, stop=sp)
                for bi, (j, loc, N, c0) in enumerate(blocks):
                    st = first and bi == 0
                    sp = last and bi == len(blocks) - 1
                    nc.tensor.matmul(o_ps[:, c0:c0 + N], v_sb[:, j, :],
                                     pp[:, loc:loc + N], start=st, stop=sp)
                if last:
                    inv = invp.tile([128, TQ], F32, tag="inv")
                    nc.vector.reciprocal_approx_fast(out=inv, in_=den)
                    nc.vector.tensor_mul(oT_sb[:, h, ti:ti + TQ], o_ps, inv)

            pend = None
            n = 0
            for h in range(QH):
                for p in range(npair):
                    j0, j1 = 2 * p, 2 * p + 1
                    t0a, N0, c0a = blk(j0)
                    t0b, N1, c0b = blk(j1)
                    sp_t = ps_s.tile([128, 2 * TQ], F32, tag="s")
                    nc.tensor.matmul(sp_t[:, 0:N0],
                                     kT_sb[:, 128 * j0:128 * (j0 + 1)],
                                     qT_sb[:, h, t0a:t0a + N0],
                                     start=True, stop=True)
                    nc.tensor.matmul(sp_t[:, TQ:TQ + N1],
                                     kT_sb[:, 128 * j1:128 * (j1 + 1)],
                                     qT_sb[:, h, t0b:t0b + N1],
                                     start=True, stop=True)
                    if j0 >= 4 * i:  # diagonal blocks: causal mask
                        nc.vector.tensor_add(sp_t[:, 0:128],
                                             sp_t[:, 0:128], cmask)
                    if j1 >= 4 * i:
                        nc.vector.tensor_add(sp_t[:, TQ:TQ + 128],
                                             sp_t[:, TQ:TQ + 128], cmask)
                    pp = ppool.tile([128, 2 * TQ], F16, tag="p")
                    ncols = TQ + N1
                    nc.scalar.activation(pp[:, :ncols], sp_t[:, :ncols],
                                         AF.Exp, scale=SCALE)
                    if pend is not None:
                        flush(*pend)
                    pend = (h, p, pp, [(j0, 0, N0, c0a), (j1, TQ, N1, c0b)])
                    n += 1
                    while fill_idx * n_items < n * len(fillers):
                        fillers[fill_idx]()
                        fill_idx += 1
            flush(*pend)
            while fill_idx < len(fillers):
                fillers[fill_idx]()
                fill_idx += 1

        def full_proj(i):
            """All projections for chunk i, V-transposes mid-way so they
            don't queue behind all the RoPE work on the DVE."""
            proj_pass(i, wk_sb, 0, HD, "k")
            proj_pass(i, wq_sb, 0, HD, "q", h=0)
            proj_pass(i, wq_sb, HD, HD, "q", h=1)
            vt = proj_pass(i, wv_sb, 0, HD, "v")
            vts(i, vt)
            proj_pass(i, wq_sb, 2 * HD, HD, "q", h=2)
            proj_pass(i, wq_sb, 3 * HD, HD, "q", h=3)

        # ======== fused pipeline ========
        # chunk 0 projections, then attn(0) filled with proj(1) k/q0 steps,
        # then the rest of proj(1), then attn(i) filled with outproj(i-1).
        full_proj(0)
        attn_chunk(0, proj_units(1, [("k", None), ("q", 0)]))
        proj_pass(1, wq_sb, HD, HD, "q", h=1)
        vt = proj_pass(1, wv_sb, 0, HD, "v")
        vts(1, vt)
        proj_pass(1, wq_sb, 2 * HD, HD, "q", h=2)
        proj_pass(1, wq_sb, 3 * HD, HD, "q", h=3)
        load_x(2)
        attn_chunk(1, outproj_units(0))
        full_proj(2)
        load_x(3)
        attn_chunk(2, outproj_units(1))
        full_proj(3)
        attn_chunk(3, outproj_units(2))
        for u in outproj_units(3):
            u()


_PERM = np.concatenate([np.arange(0, HD, 2), np.arange(1, HD, 2)])

PROFILE = False
LAST_EXEC_NS = None
LAST_RESULTS = None


def kernel(x, freqs_cos, freqs_sin, wq, wk, wv, wo):
    global LAST_EXEC_NS, LAST_RESULTS
    if "nc" not in _CACHE:
        _CACHE["nc"] = _build_nc()
    nc = _CACHE["nc"]

    x = np.asarray(x, dtype=np.float32)
    fc = np.asarray(freqs_cos, dtype=np.float32)
    fs = np.asarray(freqs_sin, dtype=np.float32)
    wq = np.asarray(wq, dtype=np.float32)
    wk = np.asarray(wk, dtype=np.float32)
    wv = np.asarray(wv, dtype=np.float32)
    wo = np.asarray(wo, dtype=np.float32)

    cosT = fc.T                                   # [64, T]
    sinT = fs.T
    cosH = np.ascontiguousarray(
        np.concatenate([cosT, cosT], axis=0).astype(np.float16))   # [128, T]
    sinH = np.ascontiguousarray(
        np.concatenate([-sinT, sinT], axis=0).astype(np.float16))

    in_maps = []
    for core in range(8):
        b, g = core // 4, core % 4
        xT = x[b].T.astype(np.float16)                        # [C, T]
        # [C, T] -> [NT, 128(p), CK(k), TQ]: xH[i, p, k, t] = xT[128k+p, 512i+t]
        xH = np.ascontiguousarray(
            xT.reshape(CK, 128, NT, TQ).transpose(2, 1, 0, 3))
        wq_g = wq[512 * g:512 * (g + 1)].reshape(QH, HD, C)[:, _PERM, :]
        wqT = wq_g.reshape(QH * HD, C).T.astype(np.float16)   # [C, 512]
        wqH = np.ascontiguousarray(
            wqT.reshape(CK, 128, QH * HD).transpose(1, 0, 2))  # [128, CK, 512]
        wkT = wk[HD * g:HD * (g + 1)][_PERM].T.astype(np.float16)  # [C, 128]
        wkH = np.ascontiguousarray(wkT.reshape(CK, 128, HD).transpose(1, 0, 2))
        wvT = wv[HD * g:HD * (g + 1)].T.astype(np.float16)
        wvH = np.ascontiguousarray(wvT.reshape(CK, 128, HD).transpose(1, 0, 2))
        wo_g = wo[:, 512 * g:512 * (g + 1)]                   # [C, 512]
        # woH[p, co, 128h+d] = wo[128co+d, 512g+128h+p]
        woH = np.ascontiguousarray(
            wo_g.reshape(CK, 128, QH, 128).transpose(3, 0, 2, 1)
        ).astype(np.float16).reshape(128, CK, QH * 128)
        in_maps.append({
            "xH": xH, "wqH": wqH, "wkH": wkH, "wvH": wvH, "woH": woH,
            "cosH": cosH, "sinH": sinH,
        })

    res = run_bass_kernel_spmd(nc, in_maps, list(range(8)), trace=PROFILE)
    LAST_EXEC_NS = res.exec_time_ns
    LAST_RESULTS = res

    out = np.empty((B, T, C), dtype=np.float32)
    for b in range(B):
        acc = res.results[4 * b]["outX"].astype(np.float32)
        for g in range(1, 4):
            acc = acc + res.results[4 * b + g]["outX"]
        # outX[i, d?, co, t]: out[b][512i+t, 128co+d] = outX[i, d, co, t]
        out[b] = acc.transpose(0, 3, 2, 1).reshape(T, C)
    return out
